# revision 24
# baseline (speedup 1.0000x reference)
"""Trainium2 Bass kernel for per-head causal attention (nn_Attention_52896817217709).

Sharding: 8 cores = 4 head-groups (3 heads each) x 2 batches.
Per core, per head h (S=2048, D_MODEL=768, D_HEAD=64):
  qT = W_Q[h].T @ Xq[h].T        (host supplies X pre-transposed: [768, 2048])
  kT, vT analogous
  S^T[k, q] = kT_chunk.T @ qT    (scores transposed: softmax-k on partitions)
  P = exp(0.125 * S^T) in fp16   (masked diagonal tile; strictly-upper tiles skipped)
  z'[d', q] = sum_k v'[k, d'].T @ P[k, q]   with v' = [v | 1] -> row 64 = softmax sums
  attn[q, m] = (z'^T_chunk.T @ [W_O; b_O/H]) * (1/sums[q])
Projections/scores/output matmuls in float32r; probability path in fp16.
The attention loop is chunk-major (one z' accumulator live) with the S^T+exp
stage running LOOKAHEAD iterations ahead of the z' matmuls so the PE never
stalls on the scalar-engine exp.
"""
import sys
import os
import numpy as np

for _p in ("/opt/trn_rl_repo", "/root/.axon_site/_ro/trn_rl_repo"):
    if os.path.isdir(_p) and _p not in sys.path:
        sys.path.insert(0, _p)

import concourse.bass as bass
import concourse.tile as tile
from concourse import bacc, mybir
from concourse.bass_utils import run_bass_kernel_spmd

F32 = mybir.dt.float32
F32R = mybir.dt.float32r
FP16 = mybir.dt.float16
AF = mybir.ActivationFunctionType

B, S, H, DM, DH = 2, 2048, 12, 768, 64
HPC = 3            # heads per core
NT = S // 128      # 16 s-tiles
MT = DM // 128     # 6 m-tiles
N_CORES = 8
LOOKAHEAD = 3      # S^T/exp stages in flight ahead of z'



def _chunks_for(i):
    """(c, qlo, w) chunks of the causal q-range [128*i, 2048) split at 512 bounds."""
    out = []
    for c in range(4):
        qlo = max(512 * c, 128 * i)
        qhi = 512 * (c + 1)
        if qhi > qlo:
            out.append((c, qlo, qhi - qlo))
    return out

def build_program():
    nc = bacc.Bacc("TRN2", target_bir_lowering=False, debug=False)

    xq = nc.dram_tensor("xq", [HPC, DM, S], FP16, kind="ExternalInput")
    xk = nc.dram_tensor("xk", [HPC, DM, S], FP16, kind="ExternalInput")
    xv = nc.dram_tensor("xv", [HPC, DM, S], FP16, kind="ExternalInput")
    wq = nc.dram_tensor("wq", [HPC, MT, 128, DH], FP16, kind="ExternalInput")
    wk = nc.dram_tensor("wk", [HPC, MT, 128, DH], FP16, kind="ExternalInput")
    wv = nc.dram_tensor("wv", [HPC, MT, 128, DH], FP16, kind="ExternalInput")
    wo = nc.dram_tensor("wo", [HPC, 128, DM], FP16, kind="ExternalInput")
    bq = nc.dram_tensor("bq", [HPC, DH, 1], F32, kind="ExternalInput")
    bk = nc.dram_tensor("bk", [HPC, DH, 1], F32, kind="ExternalInput")
    bv = nc.dram_tensor("bv", [HPC, DH, 1], F32, kind="ExternalInput")
    ident = nc.dram_tensor("ident", [128, 128], F32, kind="ExternalInput")
    identh = nc.dram_tensor("identh", [128, 128], FP16, kind="ExternalInput")
    maskd = nc.dram_tensor("maskd", [128, 128], FP16, kind="ExternalInput")
    ones16 = nc.dram_tensor("ones16", [128, NT], FP16, kind="ExternalInput")
    out = nc.dram_tensor("out", [HPC, S, DM], F32, kind="ExternalOutput")

    with tile.TileContext(nc) as tc:
        with (
            tc.tile_pool(name="wpool", bufs=1) as wpool,
            tc.tile_pool(name="xt", bufs=3) as xt_pool,
            tc.tile_pool(name="qk", bufs=3) as qk_pool,
            tc.tile_pool(name="vtp", bufs=2) as vt_pool,
            tc.tile_pool(name="vp", bufs=3) as vp_pool,
            tc.tile_pool(name="pp", bufs=3) as p_pool,
            tc.tile_pool(name="zt", bufs=2) as zt_pool,
            tc.tile_pool(name="sr", bufs=1) as sr_pool,
            tc.tile_pool(name="rc", bufs=2) as rc_pool,
            tc.tile_pool(name="ob", bufs=2) as out_pool,
            tc.tile_pool(name="ps_s", bufs=2, space="PSUM") as ps_s,
            tc.tile_pool(name="ps_a", bufs=2, space="PSUM") as ps_a,
            tc.tile_pool(name="ps_z", bufs=4, space="PSUM") as ps_z,
        ):
            id_sb = wpool.tile([128, 128], F32, name="id_sb")
            nc.gpsimd.dma_start(id_sb[:], ident[:])
            idh_sb = wpool.tile([128, 128], FP16, name="idh_sb")
            nc.gpsimd.dma_start(idh_sb[:], identh[:])
            mask_sb = wpool.tile([128, 128], FP16, name="mask_sb")
            nc.gpsimd.dma_start(mask_sb[:], maskd[:])

            st = [dict() for _ in range(HPC)]   # per-head live tiles

            TENS = {"q": (xq, wq, bq), "k": (xk, wk, bk), "v": (xv, wv, bv)}

            def emit_loads(h, order=("q", "v", "k")):
                """Big X loads on the (pure) sync ring; small DMAs on gpsimd."""
                for t in order:
                    xd, wd, bd = TENS[t]
                    xh = xt_pool.tile([128, MT, S], FP16,
                                      name=f"x{t}{h}", tag="xt")
                    for n in range(4):
                        nc.sync.dma_start(
                            xh[:, :, bass.ts(n, 512)],
                            xd[h].rearrange("(a p) s -> p a s", p=128)
                                 [:, :, bass.ts(n, 512)])
                    wt = wpool.tile([128, MT, DH], FP16, name=f"w{t}{h}")
                    nc.gpsimd.dma_start(wt[:], wd[h].rearrange("a p d -> p a d"))
                    bt = wpool.tile([DH, 1], F32, name=f"b{t}{h}")
                    nc.gpsimd.dma_start(bt[:], bd[h])
                    st[h][f"x{t}"] = xh
                    st[h][f"w{t}"] = wt
                    st[h][f"b{t}"] = bt

            def emit_proj_qv(h):
                """q and v projections col-paired on the two array halves."""
                qT = qk_pool.tile([128, S], FP16, name=f"qT{h}", tag="qT")
                nc.gpsimd.memset(qT[DH:128, :], 0.0)
                vT = vt_pool.tile([128, S], FP16, name=f"vT{h}", tag="vT")
                st[h]["qT"] = qT
                st[h]["vT"] = vT
                xq_h, wq_t, bq_t = st[h]["xq"], st[h]["wq"], st[h]["bq"]
                xv_h, wv_t, bv_t = st[h]["xv"], st[h]["wv"], st[h]["bv"]
                for c in range(4):
                    acc = ps_s.tile([128, 512], F32, name=f"accqv{h}{c}", tag="s")
                    for mt in range(MT):
                        nc.tensor.matmul(
                            acc[0:DH, :], wq_t[:, mt, :],
                            xq_h[:, mt, bass.ts(c, 512)],
                            start=(mt == 0), stop=(mt == MT - 1),
                            tile_position=(0, 0))
                        nc.tensor.matmul(
                            acc[DH:128, :], wv_t[:, mt, :],
                            xv_h[:, mt, bass.ts(c, 512)],
                            start=(mt == 0), stop=(mt == MT - 1),
                            tile_position=(0, DH))
                    nc.vector.tensor_scalar_add(
                        qT[0:DH, bass.ts(c, 512)], acc[0:DH, :], bq_t[:])
                    nc.vector.tensor_scalar_add(
                        vT[DH:128, bass.ts(c, 512)], acc[DH:128, :], bv_t[:])

            def emit_proj(h, t):
                """Single-tensor projection chains (k always; q/v for head 0)."""
                if t == "k":
                    dst = qk_pool.tile([128, S], FP16, name=f"kT{h}", tag="kT")
                    nc.gpsimd.memset(dst[DH:128, :], 0.0)
                    st[h]["kT"] = dst
                elif t == "q":
                    dst = qk_pool.tile([128, S], FP16, name=f"qT{h}", tag="qT")
                    nc.gpsimd.memset(dst[DH:128, :], 0.0)
                    st[h]["qT"] = dst
                else:
                    dst = vt_pool.tile([128, S], FP16, name=f"vT{h}", tag="vT")
                    st[h]["vT"] = dst
                xh, wt, bt = st[h][f"x{t}"], st[h][f"w{t}"], st[h][f"b{t}"]
                row0 = DH if t == "v" else 0
                tp = (0, DH) if t == "v" else None
                for cp in range(2):
                    accs = [ps_s.tile([128, 512], F32, name=f"acc{t}{h}{cp}{k}",
                                      tag="s") for k in range(2)]
                    for mt in range(MT):
                        for k in range(2):
                            nc.tensor.matmul(
                                accs[k][row0:row0 + DH, :], wt[:, mt, :],
                                xh[:, mt, bass.ts(2 * cp + k, 512)],
                                start=(mt == 0), stop=(mt == MT - 1),
                                tile_position=tp)
                    for k in range(2):
                        nc.vector.tensor_scalar_add(
                            dst[row0:row0 + DH, bass.ts(2 * cp + k, 512)],
                            accs[k][row0:row0 + DH, :], bt[:])

            def emit_vp(h):
                vT = st[h]["vT"]
                vp = vp_pool.tile([128, DH + 1, NT], FP16, name=f"vp{h}", tag="vp")
                nc.gpsimd.dma_start(vp[:, DH, :], ones16[:])
                for i in range(NT):
                    v_ps = ps_s.tile([128, DH], FP16, name=f"vps{h}{i}", tag="s",
                                     padded_shape=[128, 1024])
                    nc.tensor.transpose(v_ps[:], vT[DH:128, bass.ts(i, 128)],
                                        idh_sb[DH:128, DH:128])
                    nc.vector.tensor_copy(vp[:, 0:DH, i], v_ps[:])
                st[h]["vp"] = vp

            def emit_B(h, interleave=None):
                """Causal attention: i-major, one k-tile staged ahead.
                interleave[c] (optional) emits other work after chunk c done."""
                qT, kT, vp = st[h]["qT"], st[h]["kT"], st[h]["vp"]
                zT = zt_pool.tile([128, S], FP16, name=f"zT{h}", tag="zT")
                nc.gpsimd.memset(zT[DH:128, :], 0.0)
                srow = sr_pool.tile([DH + 1, S], F32, name=f"srow{h}", tag="srow")
                rc = rc_pool.tile([128, NT], F32, name=f"rc{h}", tag="rc")
                wot = wpool.tile([128, DM], FP16, name=f"wo{h}")
                nc.gpsimd.dma_start(wot[:], wo[h])
                st[h]["wo"] = wot

                def emit_rc(c):
                    for j in range(4 * c, 4 * c + 4):
                        rc_ps = ps_s.tile([128, 1], F32, name=f"rcp{h}{j}", tag="s")
                        nc.tensor.transpose(
                            rc_ps[:], srow[DH:DH + 1, bass.ts(j, 128)],
                            id_sb[DH:DH + 1, DH:DH + 1])
                        nc.vector.reciprocal(rc[:, j:j + 1], rc_ps[:])

                z_ps = [ps_z.tile([DH + 1, 512], F32, name=f"z{h}{c}", tag="z")
                        for c in range(4)]

                def stage_S(i):
                    """S^T matmuls (one LDW of kT_i) + exps for k-tile i."""
                    P = p_pool.tile([128, S], FP16, name=f"P{h}{i}", tag="P")
                    chs = _chunks_for(i)
                    for (c, qlo, w) in chs:
                        s_ps = ps_s.tile([128, 512], F32,
                                         name=f"s{h}{i}{c}", tag="s")
                        nc.tensor.matmul(s_ps[:, 0:w], kT[:, bass.ts(i, 128)],
                                         qT[:, qlo:qlo + w], start=True, stop=True)
                        po = qlo - 128 * i
                        nc.scalar.activation(P[:, po:po + w], s_ps[:, 0:w],
                                             AF.Exp, scale=0.125)
                        if c == chs[0][0] and qlo == 128 * i:
                            nc.vector.tensor_mul(P[:, 0:128], P[:, 0:128],
                                                 mask_sb[:])
                    return P, chs

                cur = stage_S(0)
                for i in range(NT):
                    nxt = stage_S(i + 1) if i + 1 < NT else None
                    P, chs = cur
                    for (c, qlo, w) in chs:
                        po = qlo - 128 * i
                        nc.tensor.matmul(
                            z_ps[c][:, qlo - 512 * c: qlo - 512 * c + w],
                            vp[:, :, i], P[:, po:po + w],
                            start=(i == 0), stop=(i == 4 * c + 3))
                    if i % 4 == 3:
                        c_done = (i - 3) // 4
                        nc.vector.tensor_copy(zT[0:DH + 1, bass.ts(c_done, 512)],
                                              z_ps[c_done][:])
                        nc.vector.tensor_copy(srow[DH:DH + 1, bass.ts(c_done, 512)],
                                              z_ps[c_done][DH:DH + 1, :])
                        emit_rc(c_done)
                        if interleave and c_done in interleave:
                            interleave[c_done]()
                    cur = nxt
                st[h].update(zT=zT, rc=rc)

            def emit_C(h, eighths=tuple(range(8))):
                """Output projection + per-row softmax scale + store."""
                zT, rc, wot = st[h]["zT"], st[h]["rc"], st[h]["wo"]
                for quarter in eighths:
                    ob = out_pool.tile([128, 2, DM], F32, name=f"ob{h}{quarter}",
                                       tag="ob")
                    for a in range(2):
                        j = 2 * quarter + a
                        for (mo, mw) in ((0, 512), (512, 256)):
                            a_ps = ps_a.tile([128, 512], F32,
                                             name=f"a{h}{j}{mo}", tag="a")
                            nc.tensor.matmul(a_ps[:, 0:mw],
                                             zT[:, bass.ts(j, 128)],
                                             wot[:, mo:mo + mw],
                                             start=True, stop=True)
                            if mo == 0:
                                nc.scalar.activation(ob[:, a, mo:mo + mw],
                                                     a_ps[:, 0:mw], AF.Copy,
                                                     scale=rc[:, j:j + 1])
                            else:
                                nc.vector.tensor_scalar_mul(ob[:, a, mo:mo + mw],
                                                            a_ps[:, 0:mw],
                                                            rc[:, j:j + 1])
                    nc.gpsimd.dma_start(
                        out[h, bass.ts(quarter, 256), :]
                           .rearrange("(a p) m -> p a m", p=128),
                        ob[:])

            emit_loads(0)
            emit_proj_qv(0)
            emit_proj(0, "k")
            emit_vp(0)
            for h in range(HPC):
                nxt = h + 1
                acts = {0: [], 1: [], 2: [], 3: []}
                if nxt < HPC:
                    emit_loads(nxt)
                    acts[0].append(lambda n=nxt: emit_proj_qv(n))
                    acts[1].append(lambda n=nxt: emit_proj(n, "k"))
                    acts[2].append(lambda n=nxt: emit_vp(n))
                if h >= 1:
                    acts[0].append(lambda p=h - 1: emit_C(p, (0, 1, 2, 3)))
                    acts[1].append(lambda p=h - 1: emit_C(p, (4, 5)))
                    acts[2].append(lambda p=h - 1: emit_C(p, (6, 7)))
                inter = {c: (lambda fs=fs: [f() for f in fs])
                         for c, fs in acts.items() if fs}
                emit_B(h, interleave=inter)
            emit_C(HPC - 1)
    nc.compile()
    return nc


_CACHED = None


def _program():
    global _CACHED
    if _CACHED is None:
        _CACHED = build_program()
    return _CACHED


def _make_in_maps(inputs):
    xq_f = np.asarray(inputs["normalized_resid_pre_q"], dtype=np.float32)
    xk_f = np.asarray(inputs["normalized_resid_pre_k"], dtype=np.float32)
    xv_f = np.asarray(inputs["normalized_resid_pre_v"], dtype=np.float32)
    WQ = np.asarray(inputs["W_Q"], dtype=np.float32)
    WK = np.asarray(inputs["W_K"], dtype=np.float32)
    WV = np.asarray(inputs["W_V"], dtype=np.float32)
    WO = np.asarray(inputs["W_O"], dtype=np.float32)
    bQ = np.asarray(inputs["b_Q"], dtype=np.float32)
    bK = np.asarray(inputs["b_K"], dtype=np.float32)
    bV = np.asarray(inputs["b_V"], dtype=np.float32)
    bO = np.asarray(inputs["b_O"], dtype=np.float32)

    ident = np.eye(128, dtype=np.float32)
    maskd = (np.arange(128)[:, None] <= np.arange(128)[None, :]).astype(np.float16)
    ones16 = np.ones((128, NT), np.float16)

    in_maps = []
    for c in range(N_CORES):
        b = c % 2
        hg = c // 2
        hs = slice(HPC * hg, HPC * hg + HPC)
        m = {
            "xq": np.ascontiguousarray(
                xq_f[b, :, hs, :].transpose(1, 2, 0)).astype(np.float16),
            "xk": np.ascontiguousarray(
                xk_f[b, :, hs, :].transpose(1, 2, 0)).astype(np.float16),
            "xv": np.ascontiguousarray(
                xv_f[b, :, hs, :].transpose(1, 2, 0)).astype(np.float16),
            "wq": np.ascontiguousarray(
                WQ[hs].reshape(HPC, MT, 128, DH)).astype(np.float16),
            "wk": np.ascontiguousarray(
                WK[hs].reshape(HPC, MT, 128, DH)).astype(np.float16),
            "wv": np.ascontiguousarray(
                WV[hs].reshape(HPC, MT, 128, DH)).astype(np.float16),
            "wo": np.ascontiguousarray(np.concatenate(
                [WO[hs], np.broadcast_to(bO / H, (HPC, 1, DM)),
                 np.zeros((HPC, 128 - DH - 1, DM), np.float32)],
                axis=1)).astype(np.float16),
            "bq": np.ascontiguousarray(bQ[hs].reshape(HPC, DH, 1)),
            "bk": np.ascontiguousarray(bK[hs].reshape(HPC, DH, 1)),
            "bv": np.ascontiguousarray(bV[hs].reshape(HPC, DH, 1)),
            "ident": ident,
            "identh": ident.astype(np.float16),
            "maskd": maskd,
            "ones16": ones16,
        }
        in_maps.append(m)
    return in_maps


def run(inputs, trace=False, **kw):
    nc = _program()
    in_maps = _make_in_maps(inputs)
    res = run_bass_kernel_spmd(nc, in_maps, core_ids=list(range(N_CORES)),
                               trace=trace, **kw)
    full = np.zeros((B, S, H, DM), np.float32)
    for c in range(N_CORES):
        b = c % 2
        hg = c // 2
        o = res.results[c]["out"]
        for j in range(HPC):
            full[b, :, HPC * hg + j, :] = o[j]
    return full, res


def kernel(**inputs):
    full, _ = run(inputs)
    return full
